# revision 1
# baseline (speedup 1.0000x reference)
"""Causal self-attention (B=4, T=2048, C=1024, H=16) on 8 trn2 NeuronCores.

Sharding: tensor-parallel over heads. Each core owns 2 heads:
  - Wqkv columns for its heads ([1024, 3*128] bf16, q-part pre-scaled 1/8)
  - Wproj rows for its heads ([128, 1024] bf16)
  - full x, transposed to [C, B*T] bf16 on host
Each core computes its partial projection [C, B*T] (bf16); the host sums the
8 partials in fp32 and un-transposes.

On-core dataflow, engineered to keep the PE array saturated (p-state!):
  A) QKV^T: two-pass per 512-token l-tile (qk into a 2-bank PSUM pair, then
     v), weights bf16, x bf16; q/k copied to SBUF as one [128,2,512] fp32r
     activation; v staged bf16 and PE-transposed into V natural layout.
  B) attention per (b, h, 512-wide i-tile): S^T j-tiles in 128-row pairs
     sharing a 2-bank PSUM tile, ONE exp activation per pair (bf16 out),
     causal diagonal zeroed via gpsimd affine_select, Y^T accumulated with a
     vones row giving row-sums l for free.  Softmax normalization:
     reciprocal_approx_fast (DVE) -> gpsimd partition_broadcast -> DVE mul.
  C) output projection inlined per i-tile (PSUM -> DVE copy bf16 -> DMA).
  QKV work for batch b+1 is emitted as fine-grained FILLER between attention
  j-tile pairs of batch b, so the tensor queue never drains while the scalar
  engine grinds exps.
"""

import numpy as np
from contextlib import ExitStack

import ml_dtypes

import concourse.bacc as bacc
import concourse.bass as bass
import concourse.mybir as mybir
import concourse.tile as tile
from concourse.bass_utils import run_bass_kernel_spmd

NCORES = 8
C = 1024
H = 16
D = 64                 # head dim
HPC = H // NCORES      # heads per core = 2
FPC = HPC * D          # features per core = 128
KC = C // 128          # contraction chunks = 8
SCALE = 1.0 / 8.0      # 1/sqrt(D)

F32 = mybir.dt.float32
F32R = mybir.dt.float32r
BF16 = mybir.dt.bfloat16
AF = mybir.ActivationFunctionType

_CACHE = {}
LAST_RESULT = None


def build_program(B, T):
    R = B * T
    TJ = T // 128          # 128-wide j (key) tiles per sequence = 16
    TI = T // 512          # 512-wide i (query) tiles per sequence = 4
    SB = HPC * TJ          # vaug stripes per batch = 32
    assert T % 512 == 0

    nc = bacc.Bacc("TRN2", target_bir_lowering=False, debug=False,
                   num_devices=NCORES)
    xT = nc.dram_tensor("xT", [C, R], BF16, kind="ExternalInput").ap()
    wqkv = nc.dram_tensor("wqkv", [C, 3 * FPC], BF16,
                          kind="ExternalInput").ap()
    wp = nc.dram_tensor("wp", [FPC, C], BF16, kind="ExternalInput").ap()
    ident = nc.dram_tensor("ident", [128, D], F32, kind="ExternalInput").ap()
    vones = nc.dram_tensor("vones", [128, B * SB], F32,
                           kind="ExternalInput").ap()
    ones64 = nc.dram_tensor("ones64", [128, 64], F32,
                            kind="ExternalInput").ap()
    outT = nc.dram_tensor("outT", [C, R], BF16, kind="ExternalOutput").ap()

    with tile.TileContext(nc) as tc, ExitStack() as ctx:
        const = ctx.enter_context(tc.tile_pool(name="const", bufs=1))
        big = ctx.enter_context(tc.tile_pool(name="big", bufs=1))
        xpool = ctx.enter_context(tc.tile_pool(name="xpool", bufs=24))
        vspool = ctx.enter_context(tc.tile_pool(name="vspool", bufs=2))
        ptpool = ctx.enter_context(tc.tile_pool(name="ptpool", bufs=4))
        recpool = ctx.enter_context(tc.tile_pool(name="recpool", bufs=2))
        bcpool = ctx.enter_context(tc.tile_pool(name="bcpool", bufs=2))
        rcpool = ctx.enter_context(tc.tile_pool(name="rcpool", bufs=2))
        ystpool = ctx.enter_context(tc.tile_pool(name="ystpool", bufs=2))
        opool = ctx.enter_context(tc.tile_pool(name="opool", bufs=3))
        psA = ctx.enter_context(tc.tile_pool(name="psA", bufs=1, space="PSUM"))
        psS = ctx.enter_context(tc.tile_pool(name="psS", bufs=2, space="PSUM"))
        psY = ctx.enter_context(tc.tile_pool(name="psY", bufs=3, space="PSUM"))

        # ---- constants ----
        w_sb = const.tile([128, KC, 3 * FPC], BF16)
        nc.sync.dma_start(out=w_sb,
                          in_=wqkv.rearrange("(kc p) c -> p kc c", p=128))
        wp_sb = const.tile([128, C], BF16)
        nc.sync.dma_start(out=wp_sb, in_=wp)
        identcol = const.tile([128, D], F32R)
        nc.sync.dma_start(out=identcol, in_=ident.bitcast(F32R))
        ones_sb = const.tile([128, 64], F32R)
        nc.sync.dma_start(out=ones_sb, in_=ones64.bitcast(F32R))
        # constant lower-causal mask: keep col >= row, else 0
        trimask = const.tile([128, 128], F32)
        nc.gpsimd.memset(trimask, 1.0)
        nc.gpsimd.affine_select(
            out=trimask, in_=trimask, compare_op=mybir.AluOpType.is_ge,
            fill=0.0, base=0, pattern=[[1, 128]], channel_multiplier=-1)

        # per-batch persistent tiles
        qkts, yts, vaugs = [], [], []
        for b in range(B):
            qkts.append(big.tile([128, 2, T], F32R, name=f"qkt{b}",
                                 tag=f"qkt{b}"))
            yts.append(big.tile([128, T], BF16, name=f"yt{b}", tag=f"yt{b}"))
            v = big.tile([128, D + 1, SB], F32R, name=f"va{b}", tag=f"va{b}")
            nc.sync.dma_start(out=v[:, D, :],
                              in_=vones.bitcast(F32R)[:, b * SB:(b + 1) * SB])
            vaugs.append(v)

        xT3 = xT.rearrange("(kc p) r -> p kc r", p=128)
        outTr = outT.rearrange("(cc two p) r -> cc p two r", two=2, p=128)

        # ---------- phase A unit generators (QKV projection) ----------
        def a_units(b):
            """Yield emission closures for batch b's QKV projection."""
            qkt, vaug = qkts[b], vaugs[b]
            for lt in range(TI):
                l0 = lt * 512
                r0 = b * T + l0
                xts = []

                def dma_unit(k, r0=r0, xts=xts):
                    xt = xpool.tile([128, 512], BF16, tag="xt",
                                    name=f"xt{b}_{k}")
                    nc.sync.dma_start(out=xt,
                                      in_=xT3[:, k, r0:r0 + 512])
                    xts.append(xt)

                for k in range(KC):
                    yield lambda k=k, f=dma_unit: f(k)

                vst_box = []

                # q, k, v as three single-bank accumulation passes
                for ci in range(3):
                    ps_box = []

                    def a_mm(k, ci=ci, ps_box=ps_box, xts=xts):
                        if not ps_box:
                            ps_box.append(
                                psA.tile([128, 512], F32, tag="a",
                                         name="psa"))
                        nc.tensor.matmul(
                            ps_box[0][:, :],
                            lhsT=w_sb[:, k, ci * FPC:(ci + 1) * FPC],
                            rhs=xts[k][:, :],
                            start=(k == 0), stop=(k == KC - 1),
                        )

                    for k in range(KC):
                        yield lambda k=k, f=a_mm: f(k)

                    def a_copy(ci=ci, ps_box=ps_box, l0=l0, qkt=qkt,
                               vst_box=vst_box):
                        if ci < 2:
                            nc.scalar.activation(
                                qkt[:, ci, l0:l0 + 512], ps_box[0][:, :],
                                AF.Copy)
                        else:
                            vstage = vspool.tile([128, 512], F32R, tag="vs",
                                                 name="vstage")
                            nc.scalar.activation(vstage[:, :],
                                                 ps_box[0][:, :], AF.Copy)
                            vst_box.append(vstage)

                    yield a_copy

                def v_trans(h, vst_box=vst_box, lt=lt, vaug=vaug):
                    vstage = vst_box[0]
                    ps_t = psS.tile([128, 256], F32R, tag="s", name="pst")
                    for jb in range(4):
                        # one accumulation group: start clears the whole
                        # bank, later chunks overwrite their own region
                        nc.tensor.matmul(
                            ps_t[:, jb * D:(jb + 1) * D],
                            lhsT=vstage[h * 64:(h + 1) * 64,
                                        jb * 128:(jb + 1) * 128],
                            rhs=identcol[h * 64:(h + 1) * 64, :],
                            is_transpose=True,
                            start=(jb == 0), stop=(jb == 3),
                            skip_group_check=True,
                        )
                    s0 = h * TJ + lt * 4
                    nc.vector.tensor_copy(
                        vaug[:, 0:D, s0:s0 + 4],
                        ps_t.rearrange("p (j d) -> p d j", j=4))

                for h in range(HPC):
                    yield lambda h=h, f=v_trans: f(h)

        filler = []

        def pump(n):
            for _ in range(n):
                if not filler:
                    return
                filler.pop(0)()

        # ---------- phase C unit generator (output projection) ----------
        def c_units(b, i0, yt):
            w0 = b * T + i0
            for cp in range(KC // 2):
                box = []

                def c_mm(half, cp=cp, box=box, yt=yt, i0=i0):
                    if not box:
                        box.append(psS.tile([128, 1024], F32, tag="s",
                                            name="pso"))
                    ct = 2 * cp + half
                    nc.tensor.matmul(
                        box[0][:, half * 512:half * 512 + 512],
                        lhsT=wp_sb[:, ct * 128:(ct + 1) * 128],
                        rhs=yt[:, i0:i0 + 512],
                        start=True, stop=True,
                    )

                for half in range(2):
                    yield lambda half=half, f=c_mm: f(half)

                def c_out(cp=cp, box=box, w0=w0):
                    ost = opool.tile([128, 1024], BF16, tag="o", name="ost")
                    nc.vector.tensor_copy(ost[:, :], box[0][:, :])
                    nc.sync.dma_start(
                        out=outTr[cp, :, :, w0:w0 + 512],
                        in_=ost.rearrange("p (two c) -> p two c", two=2),
                    )

                yield c_out

        # ---------- prologue: batch 0 QKV straight through ----------
        for u in a_units(0):
            u()

        # ---------- main loop ----------
        for b in range(B):
            if b + 1 < B:
                filler.extend(a_units(b + 1))
            qkt, yt, vaug = qkts[b], yts[b], vaugs[b]

            for it in range(TI):
                i0 = it * 512
                njt = (i0 + 512) // 128
                npair = njt // 2
                for h in range(HPC):
                    h0 = h * 64
                    ps_y = psY.tile([D + 1, 512], F32, tag="y", name="psy")
                    pts = [None] * npair

                    def s_pair(p, pts=pts, h0=h0, i0=i0, njt=njt):
                        ps_s = psS.tile([128, 1024], F32, tag="s", name="pss")
                        pt = ptpool.tile([128, 1024], F32R, tag="pt",
                                         name="pt")
                        for half in range(2):
                            jj = 2 * p + half
                            j0 = jj * 128
                            off = max(0, j0 - i0)
                            w = 512 - off
                            nc.tensor.matmul(
                                ps_s[:, half * 512:half * 512 + w],
                                lhsT=qkt[h0:h0 + 64, 1, j0:j0 + 128],
                                rhs=qkt[h0:h0 + 64, 0, i0 + off:i0 + 512],
                                start=True, stop=True,
                            )
                        w2 = 512 - max(0, (2 * p + 1) * 128 - i0)
                        nc.scalar.activation(pt[:, 0:512 + w2],
                                             ps_s[:, 0:512 + w2], AF.Exp)
                        for half in range(2):
                            jj = 2 * p + half
                            if jj * 128 >= i0:
                                c0 = half * 512
                                with nc.allow_low_precision(
                                        reason="exact 0/1 causal mask"):
                                    nc.vector.tensor_tensor(
                                        out=pt[:, c0:c0 + 128],
                                        in0=pt[:, c0:c0 + 128],
                                        in1=trimask[:, :].bitcast(F32R),
                                        op=mybir.AluOpType.mult,
                                    )
                        pts[p] = pt

                    def y_pair(p, pts=pts, ps_y=ps_y, h=h, i0=i0, njt=njt):
                        pt = pts[p]
                        for half in range(2):
                            jj = 2 * p + half
                            j0 = jj * 128
                            off = max(0, j0 - i0)
                            w = 512 - off
                            nc.tensor.matmul(
                                ps_y[:, off:512],
                                lhsT=vaug[:, :, h * TJ + jj],
                                rhs=pt[:, half * 512:half * 512 + w],
                                start=(jj == 0), stop=(jj == njt - 1),
                            )

                    s_pair(0)
                    for p in range(1, npair):
                        s_pair(p)
                        pump(4)
                        y_pair(p - 1)
                    pump(3)
                    y_pair(npair - 1)

                    # normalization: PE ones-broadcast of l to 64 partitions,
                    # then reciprocal_approx_fast at base partition 0
                    lsb = recpool.tile([65, 512], F32R, tag="rec", name="lsb")
                    with nc.allow_low_precision(reason="l to f32r for bcast"):
                        nc.vector.tensor_copy(lsb[64:65, :],
                                              ps_y[D:D + 1, :])
                    ps_b = psY.tile([64, 512], F32, tag="y", name="psb")
                    nc.tensor.matmul(ps_b[:, :], lhsT=ones_sb[64:65, :],
                                     rhs=lsb[64:65, :],
                                     start=True, stop=True)
                    bcl = rcpool.tile([64, 512], F32, tag="rcb", name="bcl")
                    nc.vector.tensor_copy(bcl[:, :], ps_b[:, :])
                    bc = bcpool.tile([64, 512], F32, tag="bc", name="bc")
                    nc.vector.reciprocal_approx_fast(out=bc[:, :],
                                                     in_=bcl[:, :])
                    if h == 0:
                        nc.vector.tensor_mul(yt[0:64, i0:i0 + 512],
                                             ps_y[0:D, :], bc[:, :])
                    else:
                        yst = ystpool.tile([64, 512], BF16, tag="yst",
                                           name="yst")
                        nc.vector.tensor_mul(yst[:, :], ps_y[0:D, :],
                                             bc[:, :])
                        nc.sync.dma_start(out=yt[64:128, i0:i0 + 512],
                                          in_=yst[:, :])

                # ---- phase C for this i-column: deferred via filler ----
                filler.extend(c_units(b, i0, yt))

            pump(len(filler))

    nc.compile()
    return nc


def make_in_maps(x, Wqkv, bqkv, Wproj, bproj):
    Bx, Tx, Cx = x.shape
    R = Bx * Tx
    bf = ml_dtypes.bfloat16
    xTh = np.ascontiguousarray(
        x.reshape(R, Cx).T.astype(np.float32)).astype(bf)
    eye = np.eye(D, dtype=np.float32)
    ident_h = np.ascontiguousarray(
        np.concatenate([eye, eye], axis=0))
    S = Bx * HPC * (Tx // 128)
    vones_h = np.ones((128, S), np.float32)
    # biases are zero-filled for this problem; fold a safety check anyway
    assert not np.any(bqkv) and not np.any(bproj), \
        "nonzero biases unsupported in this build"
    in_maps = []
    for i in range(NCORES):
        cs = slice(i * FPC, (i + 1) * FPC)
        wq = Wqkv[:, 0 * C:1 * C][:, cs] * SCALE
        wk = Wqkv[:, 1 * C:2 * C][:, cs]
        wv = Wqkv[:, 2 * C:3 * C][:, cs]
        wqkv_s = np.ascontiguousarray(
            np.concatenate([wq, wk, wv], axis=1).astype(np.float32)).astype(bf)
        wp_s = np.ascontiguousarray(Wproj[cs, :].astype(np.float32)).astype(bf)
        in_maps.append({
            "xT": xTh,
            "wqkv": wqkv_s,
            "wp": wp_s,
            "ident": ident_h,
            "vones": vones_h,
            "ones64": np.ones((128, 64), np.float32),
        })
    return in_maps


def kernel(x, Wqkv, bqkv, Wproj, bproj, trace=False):
    global LAST_RESULT
    x = np.asarray(x, dtype=np.float32)
    Wqkv = np.asarray(Wqkv, dtype=np.float32)
    bqkv = np.asarray(bqkv, dtype=np.float32)
    Wproj = np.asarray(Wproj, dtype=np.float32)
    bproj = np.asarray(bproj, dtype=np.float32)
    Bx, Tx, Cx = x.shape
    assert Cx == C

    key = (Bx, Tx)
    if key not in _CACHE:
        _CACHE[key] = build_program(Bx, Tx)
    nc = _CACHE[key]

    in_maps = make_in_maps(x, Wqkv, bqkv, Wproj, bproj)
    res = run_bass_kernel_spmd(nc, in_maps, list(range(NCORES)), trace=trace)
    LAST_RESULT = res
    acc = np.zeros((C, Bx * Tx), dtype=np.float32)
    for i in range(NCORES):
        acc += res.results[i]["outT"].astype(np.float32)
    return np.ascontiguousarray(acc.T).reshape(Bx, Tx, Cx)



# revision 13
# speedup vs baseline: 1.2307x; 1.2307x over previous
"""Causal self-attention (B=4, T=2048, C=1024, H=16) on 8 trn2 NeuronCores.

Sharding: 4-way tensor parallel over heads x 2-way data parallel over batch.
Core i handles head group tp = i % 4 (4 heads) for batches [2*dp, 2*dp+1],
dp = i // 4.  Each core:
  - Wqkv columns for its 4 heads ([1024, 768] bf16, q-part pre-scaled 1/8)
  - Wproj rows for its heads ([256, 1024] bf16)
  - x for its 2 batches, transposed to [C, 2T] bf16 on host
Each core computes a partial projection [C, 2T] bf16; the host sums the 4
TP partials per batch pair in fp32 and un-transposes.

All PE operands are bf16 (f32r streams ~1.5-2x slower on HW and fp32
weight loads are 2x slower).  On-core dataflow:
  A) QKV^T per 512-token l-tile: 6 single-bank accumulation passes
     (q/k/v x 2 head-groups); q/k copied to SBUF bf16; v staged bf16 and
     PE-transposed (2 heads per transpose) into V-natural layout vaug
     with a trailing ones row (row sums l for free in PV).
  B) attention per (b, h, 512-wide i-tile): S^T j-tile pairs share a
     2-bank PSUM tile and a common query window (both halves start at the
     even tile's clip offset), ONE exp per pair (bf16 out), causal masking
     via two small bf16 mask multiplies on DVE, Y^T accumulated per j-tile.
     Softmax normalization: ACT copies l to SBUF, DVE fast reciprocal,
     gpsimd partition_broadcast to 64 lanes, one DVE multiply.
  C) output projection per i-tile (2-pass K=256 accumulation, bf16 copy
     out on DVE -> DMA).
  QKV work for batch b+1 and projection work are emitted as fine-grained
  FILLER between attention pairs so the tensor queue never drains.
"""

import numpy as np
from contextlib import ExitStack

import ml_dtypes

import concourse.bacc as bacc
import concourse.bass as bass
import concourse.mybir as mybir
import concourse.tile as tile
from concourse.bass_utils import run_bass_kernel_spmd

NCORES = 8
TPC = 4                # tensor-parallel cores (head groups)
DPC = 2                # data-parallel groups
C = 1024
H = 16
D = 64                 # head dim
HPC = H // TPC         # heads per core = 4
HG = HPC // 2          # head groups of 2 per core = 2
FPC = HPC * D          # features per core = 256
KC = C // 128          # contraction chunks = 8
SCALE = 1.0 / 8.0      # 1/sqrt(D)

F32 = mybir.dt.float32
BF16 = mybir.dt.bfloat16
AF = mybir.ActivationFunctionType

_CACHE = {}
LAST_RESULT = None


def build_program(B, T):
    BL = B // DPC          # local batches = 2
    R = BL * T             # local tokens = 4096
    TJ = T // 128          # 128-wide j (key) tiles per sequence = 16
    TI = T // 512          # 512-wide i (query) tiles per sequence = 4
    SB = HPC * TJ          # vaug stripes per batch = 64
    assert T % 512 == 0

    nc = bacc.Bacc("TRN2", target_bir_lowering=False, debug=False,
                   num_devices=NCORES)
    xT = nc.dram_tensor("xT", [C, R], BF16, kind="ExternalInput").ap()
    wqkv = nc.dram_tensor("wqkv", [C, 3 * FPC], BF16,
                          kind="ExternalInput").ap()
    wp = nc.dram_tensor("wp", [FPC, C], BF16, kind="ExternalInput").ap()
    ident = nc.dram_tensor("ident", [128, 128], BF16,
                           kind="ExternalInput").ap()
    ones64 = nc.dram_tensor("ones64", [65, 64], F32,
                            kind="ExternalInput").ap()
    vones = nc.dram_tensor("vones", [128, BL * SB], BF16,
                           kind="ExternalInput").ap()
    # masks[:, 0:128]  = tril (keep col >= row)
    # masks[:, 128:384] = keep col-128 >= row (odd half of a pair window)
    masks = nc.dram_tensor("masks", [128, 384], BF16,
                           kind="ExternalInput").ap()
    outT = nc.dram_tensor("outT", [C, R], BF16, kind="ExternalOutput").ap()

    with tile.TileContext(nc) as tc, ExitStack() as ctx:
        const = ctx.enter_context(tc.tile_pool(name="const", bufs=1))
        big = ctx.enter_context(tc.tile_pool(name="big", bufs=1))
        xpool = ctx.enter_context(tc.tile_pool(name="xpool", bufs=24))
        vspool = ctx.enter_context(tc.tile_pool(name="vspool", bufs=2))
        ptpool = ctx.enter_context(tc.tile_pool(name="ptpool", bufs=4))
        lpool = ctx.enter_context(tc.tile_pool(name="lpool", bufs=2))
        rpool = ctx.enter_context(tc.tile_pool(name="rpool", bufs=2))
        bcpool = ctx.enter_context(tc.tile_pool(name="bcpool", bufs=2))
        ystpool = ctx.enter_context(tc.tile_pool(name="ystpool", bufs=2))
        opool = ctx.enter_context(tc.tile_pool(name="opool", bufs=3))
        psA = ctx.enter_context(tc.tile_pool(name="psA", bufs=1, space="PSUM"))
        psS = ctx.enter_context(tc.tile_pool(name="psS", bufs=2, space="PSUM"))
        psY = ctx.enter_context(tc.tile_pool(name="psY", bufs=3, space="PSUM"))

        # ---- constants ----
        w_sb = const.tile([128, KC, 3 * FPC], BF16)
        nc.sync.dma_start(out=w_sb,
                          in_=wqkv.rearrange("(kc p) c -> p kc c", p=128))
        wp_sb = const.tile([128, 2, C], BF16)
        nc.sync.dma_start(out=wp_sb,
                          in_=wp.rearrange("(kc p) c -> p kc c", p=128))
        identcol = const.tile([128, 128], BF16)
        nc.sync.dma_start(out=identcol, in_=ident)
        F32R = mybir.dt.float32r
        ones_sb = const.tile([65, 64], F32R)
        nc.sync.dma_start(out=ones_sb, in_=ones64.bitcast(F32R))
        mask_sb = const.tile([128, 384], BF16)
        nc.sync.dma_start(out=mask_sb, in_=masks)

        # per-batch persistent tiles
        qkts, yts, vaugs = [], [], []
        for b in range(BL):
            qkts.append(big.tile([128, HG, 2, T], BF16, name=f"qkt{b}",
                                 tag=f"qkt{b}"))
            yts.append(big.tile([128, 2, T], BF16, name=f"yt{b}",
                                tag=f"yt{b}"))
            v = big.tile([128, D + 1, SB], BF16, name=f"va{b}", tag=f"va{b}")
            nc.sync.dma_start(out=v[:, D, :],
                              in_=vones[:, b * SB:(b + 1) * SB])
            vaugs.append(v)

        xT3 = xT.rearrange("(kc p) r -> p kc r", p=128)
        outTr = outT.rearrange("(cc two p) r -> cc p two r", two=2, p=128)

        # ---------- phase A unit generators (QKV projection) ----------
        def a_units(b):
            """Yield emission closures for batch b's QKV projection."""
            qkt, vaug = qkts[b], vaugs[b]
            for lt in range(TI):
                l0 = lt * 512
                r0 = b * T + l0
                xts = []

                def dma_unit(k, r0=r0, xts=xts):
                    xt = xpool.tile([128, 512], BF16, tag="xt",
                                    name=f"xt{b}_{k}")
                    nc.sync.dma_start(out=xt,
                                      in_=xT3[:, k, r0:r0 + 512])
                    xts.append(xt)

                for k in range(KC):
                    yield lambda k=k, f=dma_unit: f(k)

                # ci: 0..3 = q/k for head groups, 4..5 = v head groups
                for ci in range(6):
                    ps_box = []

                    def a_mm(k, ci=ci, ps_box=ps_box, xts=xts):
                        if not ps_box:
                            ps_box.append(
                                psA.tile([128, 512], F32, tag="a",
                                         name="psa"))
                        nc.tensor.matmul(
                            ps_box[0][:, :],
                            lhsT=w_sb[:, k, ci * 128:(ci + 1) * 128],
                            rhs=xts[k][:, :],
                            start=(k == 0), stop=(k == KC - 1),
                        )

                    for k in range(KC):
                        yield lambda k=k, f=a_mm: f(k)

                    if ci < 4:
                        # q/k: ci -> (kind=ci//2, hg=ci%2)
                        def a_copy(ci=ci, ps_box=ps_box, l0=l0, qkt=qkt):
                            qk, hg = ci // 2, ci % 2
                            nc.scalar.activation(
                                qkt[:, hg, qk, l0:l0 + 512], ps_box[0][:, :],
                                AF.Copy)

                        yield a_copy
                    else:
                        # v head-group hg = ci-4: stage, transpose, store
                        def v_stage(ci=ci, ps_box=ps_box, lt=lt, vaug=vaug):
                            hg = ci - 4
                            vstage = vspool.tile([128, 512], BF16, tag="vs",
                                                 name="vstage")
                            nc.scalar.activation(vstage[:, :],
                                                 ps_box[0][:, :], AF.Copy)
                            ps_t = psA.tile([128, 512], BF16, tag="a",
                                            name="pst")
                            for jb in range(4):
                                nc.tensor.matmul(
                                    ps_t[:, jb * 128:(jb + 1) * 128],
                                    lhsT=vstage[:, jb * 128:(jb + 1) * 128],
                                    rhs=identcol[:, :],
                                    is_transpose=True,
                                    start=(jb == 0), stop=(jb == 3),
                                    skip_group_check=True,
                                )
                            # stripes: head h=2*hg+hh at (2*hg+hh)*TJ + lt*4
                            va_r = vaug[:, 0:D, :].rearrange(
                                "p d (h t j) -> p d h t j",
                                h=HPC, t=TI, j=4)
                            nc.vector.tensor_copy(
                                va_r[:, :, 2 * hg:2 * hg + 2, lt, :],
                                ps_t.rearrange("p (jb hh d) -> p d hh jb",
                                               jb=4, hh=2))

                        yield v_stage

        filler = []

        def pump(n):
            for _ in range(n):
                if not filler:
                    return
                filler.pop(0)()

        # ---------- phase C unit generator (output projection) ----------
        def c_units(b, i0, yt):
            w0 = b * T + i0
            for cp in range(KC // 2):
                box = []

                def c_mm(half, kc, cp=cp, box=box, yt=yt, i0=i0):
                    if not box:
                        box.append(psS.tile([128, 1024], F32, tag="s",
                                            name="pso"))
                    ct = 2 * cp + half
                    nc.tensor.matmul(
                        box[0][:, half * 512:half * 512 + 512],
                        lhsT=wp_sb[:, kc, ct * 128:(ct + 1) * 128],
                        rhs=yt[:, kc, i0:i0 + 512],
                        start=(kc == 0), stop=(kc == 1),
                    )

                for half in range(2):
                    for kc in range(2):
                        yield lambda half=half, kc=kc, f=c_mm: f(half, kc)

                def c_out(cp=cp, box=box, w0=w0):
                    ost = opool.tile([128, 1024], BF16, tag="o", name="ost")
                    nc.vector.tensor_copy(ost[:, :], box[0][:, :])
                    nc.sync.dma_start(
                        out=outTr[cp, :, :, w0:w0 + 512],
                        in_=ost.rearrange("p (two c) -> p two c", two=2),
                    )

                yield c_out

        # ---------- prologue: batch 0 QKV straight through ----------
        for u in a_units(0):
            u()

        # ---------- main loop ----------
        for b in range(BL):
            if b + 1 < BL:
                filler.extend(a_units(b + 1))
            qkt, yt, vaug = qkts[b], yts[b], vaugs[b]

            for it in range(TI):
                i0 = it * 512
                njt = (i0 + 512) // 128
                npair = njt // 2
                for h in range(HPC):
                    hg, hh = h // 2, h % 2
                    p0 = 64 * hh
                    ps_y = psY.tile([D + 1, 512], F32, tag="y", name="psy")
                    pts = [None] * npair

                    def s_pair(p, pts=pts, hg=hg, p0=p0, i0=i0):
                        # both halves share the even tile's query window
                        off = max(0, 2 * p * 128 - i0)
                        w = 512 - off
                        straddle = (2 * p + 1) * 128 > i0
                        ps_s = psS.tile([128, 1024], F32, tag="s", name="pss")
                        pt = ptpool.tile([128, 1024], BF16, tag="pt",
                                         name="pt")
                        for half in range(2):
                            j0 = (2 * p + half) * 128
                            nc.tensor.matmul(
                                ps_s[:, half * 512:half * 512 + w],
                                lhsT=qkt[p0:p0 + 64, hg, 1, j0:j0 + 128],
                                rhs=qkt[p0:p0 + 64, hg, 0,
                                        i0 + off:i0 + 512],
                                start=True, stop=True,
                            )
                        if w == 512:
                            nc.scalar.activation(pt[:, 0:1024],
                                                 ps_s[:, 0:1024], AF.Exp)
                        else:
                            nc.scalar.activation(pt[:, 0:w],
                                                 ps_s[:, 0:w], AF.Exp)
                            nc.scalar.activation(pt[:, 512:512 + w],
                                                 ps_s[:, 512:512 + w], AF.Exp)
                        if straddle:
                            mw = min(w, 256)
                            nc.vector.tensor_tensor(
                                out=pt[:, 0:128], in0=pt[:, 0:128],
                                in1=mask_sb[:, 0:128],
                                op=mybir.AluOpType.mult)
                            nc.vector.tensor_tensor(
                                out=pt[:, 512:512 + mw],
                                in0=pt[:, 512:512 + mw],
                                in1=mask_sb[:, 128:128 + mw],
                                op=mybir.AluOpType.mult)
                        pts[p] = pt

                    def y_pair(p, pts=pts, ps_y=ps_y, h=h, i0=i0, njt=njt):
                        pt = pts[p]
                        off = max(0, 2 * p * 128 - i0)
                        w = 512 - off
                        for half in range(2):
                            jj = 2 * p + half
                            nc.tensor.matmul(
                                ps_y[:, off:512],
                                lhsT=vaug[:, :, h * TJ + jj],
                                rhs=pt[:, half * 512:half * 512 + w],
                                start=(jj == 0), stop=(jj == njt - 1),
                            )

                    s_pair(0)
                    for p in range(1, npair):
                        s_pair(p)
                        pump(4)
                        y_pair(p - 1)
                    pump(3)
                    y_pair(npair - 1)

                    # normalization: l -> SBUF f32r (DVE), PE ones-broadcast
                    # to 64 lanes, copy out (ACT), reciprocal + multiply (DVE)
                    lsb = lpool.tile([D + 1, 512], F32R, tag="l", name="lsb")
                    with nc.allow_low_precision(reason="l to f32r for bcast"):
                        nc.vector.tensor_copy(lsb[D:D + 1, :],
                                              ps_y[D:D + 1, :])
                    ps_b = psY.tile([64, 512], F32, tag="y", name="psb")
                    nc.tensor.matmul(ps_b[:, :], lhsT=ones_sb[64:65, :],
                                     rhs=lsb[D:D + 1, :],
                                     start=True, stop=True)
                    bcl = rpool.tile([64, 512], F32, tag="r", name="bcl")
                    nc.scalar.activation(bcl[:, :], ps_b[:, :], AF.Copy)
                    bc = bcpool.tile([64, 512], F32, tag="bc", name="bc")
                    nc.vector.reciprocal_approx_fast(out=bc[:, :],
                                                     in_=bcl[:, :])
                    if hh == 0:
                        nc.vector.tensor_mul(yt[0:64, hg, i0:i0 + 512],
                                             ps_y[0:D, :], bc[:, :])
                    else:
                        yst = ystpool.tile([64, 512], BF16, tag="yst",
                                           name="yst")
                        nc.vector.tensor_mul(yst[:, :], ps_y[0:D, :],
                                             bc[:, :])
                        nc.sync.dma_start(out=yt[64:128, hg, i0:i0 + 512],
                                          in_=yst[:, :])

                # ---- phase C for this i-column: deferred via filler ----
                filler.extend(c_units(b, i0, yt))

            pump(len(filler))

    nc.compile()
    return nc


def make_in_maps(x, Wqkv, bqkv, Wproj, bproj):
    Bx, Tx, Cx = x.shape
    bf = ml_dtypes.bfloat16
    BL = Bx // DPC
    R = BL * Tx
    # per-dp-group transposed activations
    xTh = []
    for dp in range(DPC):
        xg = x[dp * BL:(dp + 1) * BL].reshape(R, Cx)
        xTh.append(np.ascontiguousarray(
            xg.T.astype(np.float32)).astype(bf))
    ident_h = np.eye(128, dtype=np.float32)
    ones64_h = np.ones((65, 64), np.float32)
    S = BL * HPC * (Tx // 128)
    vones_h = np.ones((128, S), np.float32)
    m128 = np.triu(np.ones((128, 128), np.float32))
    m256 = np.zeros((128, 256), np.float32)
    for r in range(128):
        m256[r, 128 + r:] = 1.0
    masks_h = np.concatenate([m128, m256], axis=1).astype(bf)
    assert not np.any(bqkv) and not np.any(bproj), \
        "nonzero biases unsupported in this build"
    in_maps = []
    for i in range(NCORES):
        tp, dp = i % TPC, i // TPC
        cs = slice(tp * FPC, (tp + 1) * FPC)
        wq = Wqkv[:, 0 * C:1 * C][:, cs] * SCALE
        wk = Wqkv[:, 1 * C:2 * C][:, cs]
        wv = Wqkv[:, 2 * C:3 * C][:, cs]
        # ci order: q_hg0, q_hg1, k_hg0, k_hg1, v_hg0, v_hg1
        wqkv_s = np.ascontiguousarray(np.concatenate(
            [wq[:, 0:128], wq[:, 128:256],
             wk[:, 0:128], wk[:, 128:256],
             wv[:, 0:128], wv[:, 128:256]], axis=1)
            .astype(np.float32)).astype(bf)
        wp_s = np.ascontiguousarray(Wproj[cs, :].astype(np.float32)).astype(bf)
        in_maps.append({
            "xT": xTh[dp],
            "wqkv": wqkv_s,
            "wp": wp_s,
            "ident": ident_h.astype(bf),
            "ones64": ones64_h,
            "vones": vones_h.astype(bf),
            "masks": masks_h,
        })
    return in_maps


def kernel(x, Wqkv, bqkv, Wproj, bproj, trace=False):
    global LAST_RESULT
    x = np.asarray(x, dtype=np.float32)
    Wqkv = np.asarray(Wqkv, dtype=np.float32)
    bqkv = np.asarray(bqkv, dtype=np.float32)
    Wproj = np.asarray(Wproj, dtype=np.float32)
    bproj = np.asarray(bproj, dtype=np.float32)
    Bx, Tx, Cx = x.shape
    assert Cx == C

    key = (Bx, Tx)
    if key not in _CACHE:
        _CACHE[key] = build_program(Bx, Tx)
    nc = _CACHE[key]

    in_maps = make_in_maps(x, Wqkv, bqkv, Wproj, bproj)
    res = run_bass_kernel_spmd(nc, in_maps, list(range(NCORES)), trace=trace)
    LAST_RESULT = res
    BL = Bx // DPC
    out = np.empty((Bx, Tx, Cx), dtype=np.float32)
    for dp in range(DPC):
        acc = np.zeros((C, BL * Tx), dtype=np.float32)
        for tp in range(TPC):
            acc += res.results[dp * TPC + tp]["outT"].astype(np.float32)
        out[dp * BL:(dp + 1) * BL] = \
            np.ascontiguousarray(acc.T).reshape(BL, Tx, Cx)
    return out


# revision 16
# speedup vs baseline: 1.2563x; 1.0209x over previous
"""Causal self-attention (B=4, T=2048, C=1024, H=16) on 8 trn2 NeuronCores.

Sharding: 4-way tensor parallel over heads x 2-way data parallel over batch.
Core i handles head group tp = i % 4 (4 heads) for batches [2*dp, 2*dp+1],
dp = i // 4.  Each core:
  - Wqkv columns for its 4 heads ([1024, 768] bf16, q-part pre-scaled 1/8)
  - Wproj rows for its heads ([256, 1024] bf16)
  - x for its 2 batches, transposed to [C, 2T] bf16 on host
Each core computes a partial projection [C, 2T] bf16; the host sums the 4
TP partials per batch pair in fp32 and un-transposes.

All PE operands are bf16 (f32r streams ~1.5-2x slower on HW and fp32
weight loads are 2x slower).  On-core dataflow:
  A) QKV^T per 512-token l-tile: 6 single-bank accumulation passes
     (q/k/v x 2 head-groups); q/k copied to SBUF bf16; v staged bf16 and
     PE-transposed (2 heads per transpose) into V-natural layout vaug
     with a trailing ones row (row sums l for free in PV).
  B) attention per (b, h, 512-wide i-tile): S^T j-tile pairs share a
     2-bank PSUM tile and a common query window (both halves start at the
     even tile's clip offset), ONE exp per pair (bf16 out), causal masking
     via two small bf16 mask multiplies on DVE, Y^T accumulated per j-tile.
     Softmax normalization: ACT copies l to SBUF, DVE fast reciprocal,
     gpsimd partition_broadcast to 64 lanes, one DVE multiply.
  C) output projection per i-tile (2-pass K=256 accumulation, bf16 copy
     out on DVE -> DMA).
  QKV work for batch b+1 and projection work are emitted as fine-grained
  FILLER between attention pairs so the tensor queue never drains.
"""

import numpy as np
from contextlib import ExitStack

import ml_dtypes

import concourse.bacc as bacc
import concourse.bass as bass
import concourse.mybir as mybir
import concourse.tile as tile
from concourse.bass_utils import run_bass_kernel_spmd

NCORES = 8
TPC = 4                # tensor-parallel cores (head groups)
DPC = 2                # data-parallel groups
C = 1024
H = 16
D = 64                 # head dim
HPC = H // TPC         # heads per core = 4
HG = HPC // 2          # head groups of 2 per core = 2
FPC = HPC * D          # features per core = 256
KC = C // 128          # contraction chunks = 8
SCALE = 1.0 / 8.0      # 1/sqrt(D)

F32 = mybir.dt.float32
BF16 = mybir.dt.bfloat16
AF = mybir.ActivationFunctionType

_CACHE = {}
LAST_RESULT = None


def build_program(B, T):
    BL = B // DPC          # local batches = 2
    R = BL * T             # local tokens = 4096
    TJ = T // 128          # 128-wide j (key) tiles per sequence = 16
    TI = T // 512          # 512-wide i (query) tiles per sequence = 4
    SB = HPC * TJ          # vaug stripes per batch = 64
    assert T % 512 == 0

    nc = bacc.Bacc("TRN2", target_bir_lowering=False, debug=False,
                   num_devices=NCORES)
    xT = nc.dram_tensor("xT", [C, R], BF16, kind="ExternalInput").ap()
    wqkv = nc.dram_tensor("wqkv", [C, 3 * FPC], BF16,
                          kind="ExternalInput").ap()
    wp = nc.dram_tensor("wp", [FPC, C], BF16, kind="ExternalInput").ap()
    ident = nc.dram_tensor("ident", [128, 128], BF16,
                           kind="ExternalInput").ap()
    ones64 = nc.dram_tensor("ones64", [65, 64], F32,
                            kind="ExternalInput").ap()
    vones = nc.dram_tensor("vones", [128, BL * SB], BF16,
                           kind="ExternalInput").ap()
    # masks[:, 0:128]  = tril (keep col >= row)
    # masks[:, 128:384] = keep col-128 >= row (odd half of a pair window)
    masks = nc.dram_tensor("masks", [128, 384], BF16,
                           kind="ExternalInput").ap()
    outT = nc.dram_tensor("outT", [C, R], BF16, kind="ExternalOutput").ap()

    with tile.TileContext(nc) as tc, ExitStack() as ctx:
        const = ctx.enter_context(tc.tile_pool(name="const", bufs=1))
        big = ctx.enter_context(tc.tile_pool(name="big", bufs=1))
        xpool = ctx.enter_context(tc.tile_pool(name="xpool", bufs=32))
        vspool = ctx.enter_context(tc.tile_pool(name="vspool", bufs=2))
        ptpool = ctx.enter_context(tc.tile_pool(name="ptpool", bufs=4))
        lpool = ctx.enter_context(tc.tile_pool(name="lpool", bufs=2))
        rpool = ctx.enter_context(tc.tile_pool(name="rpool", bufs=2))
        bcpool = ctx.enter_context(tc.tile_pool(name="bcpool", bufs=2))
        ystpool = ctx.enter_context(tc.tile_pool(name="ystpool", bufs=2))
        opool = ctx.enter_context(tc.tile_pool(name="opool", bufs=3))
        psA = ctx.enter_context(tc.tile_pool(name="psA", bufs=1, space="PSUM"))
        psS = ctx.enter_context(tc.tile_pool(name="psS", bufs=2, space="PSUM"))
        psY = ctx.enter_context(tc.tile_pool(name="psY", bufs=3, space="PSUM"))

        # ---- constants ----
        w_sb = const.tile([128, KC, 3 * FPC], BF16)
        nc.sync.dma_start(out=w_sb,
                          in_=wqkv.rearrange("(kc p) c -> p kc c", p=128))
        wp_sb = const.tile([128, 2, C], BF16)
        nc.sync.dma_start(out=wp_sb,
                          in_=wp.rearrange("(kc p) c -> p kc c", p=128))
        identcol = const.tile([128, 128], BF16)
        nc.sync.dma_start(out=identcol, in_=ident)
        F32R = mybir.dt.float32r
        ones_sb = const.tile([65, 64], F32R)
        nc.sync.dma_start(out=ones_sb, in_=ones64.bitcast(F32R))
        mask_sb = const.tile([128, 384], BF16)
        nc.sync.dma_start(out=mask_sb, in_=masks)

        # per-batch persistent tiles
        qkts, yts, vaugs = [], [], []
        for b in range(BL):
            qkts.append(big.tile([128, HG, 2, T], BF16, name=f"qkt{b}",
                                 tag=f"qkt{b}"))
            yts.append(big.tile([128, 2, T], BF16, name=f"yt{b}",
                                tag=f"yt{b}"))
            v = big.tile([128, D + 1, SB], BF16, name=f"va{b}", tag=f"va{b}")
            nc.sync.dma_start(out=v[:, D, :],
                              in_=vones[:, b * SB:(b + 1) * SB])
            vaugs.append(v)

        xT3 = xT.rearrange("(kc p) r -> p kc r", p=128)
        outTr = outT.rearrange("(cc two p) r -> cc p two r", two=2, p=128)

        # ---------- phase A unit generators (QKV projection) ----------
        xts_store = {}

        def ci_units(b, lt, ci):
            """8 accumulation matmuls + 1 epilogue unit for one 128-col
            weight slice of l-tile (b, lt)."""
            qkt, vaug = qkts[b], vaugs[b]
            l0 = lt * 512
            xts = xts_store[(b, lt)]
            ps_box = []

            def a_mm(k, ci=ci, ps_box=ps_box, xts=xts):
                if not ps_box:
                    ps_box.append(
                        psA.tile([128, 512], F32, tag="a", name="psa"))
                nc.tensor.matmul(
                    ps_box[0][:, :],
                    lhsT=w_sb[:, k, ci * 128:(ci + 1) * 128],
                    rhs=xts[k][:, :],
                    start=(k == 0), stop=(k == KC - 1),
                )

            for k in range(KC):
                yield lambda k=k, f=a_mm: f(k)

            if ci < 4:
                # q/k: ci -> (kind=ci//2, hg=ci%2)
                def a_copy(ci=ci, ps_box=ps_box, l0=l0, qkt=qkt):
                    qk, hg = ci // 2, ci % 2
                    nc.scalar.activation(
                        qkt[:, hg, qk, l0:l0 + 512], ps_box[0][:, :],
                        AF.Copy)

                yield a_copy
            else:
                # v head-group hg = ci-4: stage, transpose, store
                def v_stage(ci=ci, ps_box=ps_box, lt=lt, vaug=vaug):
                    hg = ci - 4
                    vstage = vspool.tile([128, 512], BF16, tag="vs",
                                         name="vstage")
                    nc.scalar.activation(vstage[:, :],
                                         ps_box[0][:, :], AF.Copy)
                    ps_t = psA.tile([128, 512], BF16, tag="a",
                                    name="pst")
                    for jb in range(4):
                        nc.tensor.matmul(
                            ps_t[:, jb * 128:(jb + 1) * 128],
                            lhsT=vstage[:, jb * 128:(jb + 1) * 128],
                            rhs=identcol[:, :],
                            is_transpose=True,
                            start=(jb == 0), stop=(jb == 3),
                            skip_group_check=True,
                        )
                    # stripes: head h=2*hg+hh at (2*hg+hh)*TJ + lt*4
                    va_r = vaug[:, 0:D, :].rearrange(
                        "p d (h t j) -> p d h t j",
                        h=HPC, t=TI, j=4)
                    nc.vector.tensor_copy(
                        va_r[:, :, 2 * hg:2 * hg + 2, lt, :],
                        ps_t.rearrange("p (jb hh d) -> p d hh jb",
                                       jb=4, hh=2))

                yield v_stage

        def v_units(b, lt):
            for ci in (4, 5):
                yield from ci_units(b, lt, ci)

        def a_units(b, defer_v=False):
            """Yield emission closures for batch b's QKV projection."""
            for lt in range(TI):
                r0 = b * T + lt * 512
                xts = xts_store.setdefault((b, lt), [])

                def dma_unit(k, r0=r0, xts=xts):
                    xt = xpool.tile([128, 512], BF16, tag="xt",
                                    name=f"xt{b}_{k}")
                    nc.sync.dma_start(out=xt,
                                      in_=xT3[:, k, r0:r0 + 512])
                    xts.append(xt)

                for k in range(KC):
                    yield lambda k=k, f=dma_unit: f(k)

                for ci in range(4):
                    yield from ci_units(b, lt, ci)
                if not defer_v:
                    yield from v_units(b, lt)

        filler = []

        def pump(n):
            for _ in range(n):
                if not filler:
                    return
                filler.pop(0)()

        # ---------- phase C unit generator (output projection) ----------
        def c_units(b, i0, yt):
            w0 = b * T + i0
            for cp in range(KC // 2):
                box = []

                def c_mm(half, kc, cp=cp, box=box, yt=yt, i0=i0):
                    if not box:
                        box.append(psS.tile([128, 1024], F32, tag="s",
                                            name="pso"))
                    ct = 2 * cp + half
                    nc.tensor.matmul(
                        box[0][:, half * 512:half * 512 + 512],
                        lhsT=wp_sb[:, kc, ct * 128:(ct + 1) * 128],
                        rhs=yt[:, kc, i0:i0 + 512],
                        start=(kc == 0), stop=(kc == 1),
                    )

                for half in range(2):
                    for kc in range(2):
                        yield lambda half=half, kc=kc, f=c_mm: f(half, kc)

                def c_out(cp=cp, box=box, w0=w0):
                    ost = opool.tile([128, 1024], BF16, tag="o", name="ost")
                    nc.vector.tensor_copy(ost[:, :], box[0][:, :])
                    nc.sync.dma_start(
                        out=outTr[cp, :, :, w0:w0 + 512],
                        in_=ost.rearrange("p (two c) -> p two c", two=2),
                    )

                yield c_out

        # ---------- phase B stream factory ----------
        def make_stream(b, it, h):
            qkt, yt, vaug = qkts[b], yts[b], vaugs[b]
            i0 = it * 512
            njt = (i0 + 512) // 128
            npair = njt // 2
            hg, hh = h // 2, h % 2
            p0 = 64 * hh
            ps_y = psY.tile([D + 1, 512], F32, tag="y", name="psy")
            pts = [None] * npair

            def s_pair(p):
                # both halves share the even tile's query window; the odd
                # strip is laid contiguously at [w, 2w) so one exp covers
                # exactly the written region
                off = max(0, 2 * p * 128 - i0)
                w = 512 - off
                straddle = (2 * p + 1) * 128 > i0
                ps_s = psS.tile([128, 1024], F32, tag="s", name="pss")
                pt = ptpool.tile([128, 1024], BF16, tag="pt", name="pt")
                for half in range(2):
                    j0 = (2 * p + half) * 128
                    nc.tensor.matmul(
                        ps_s[:, half * w:half * w + w],
                        lhsT=qkt[p0:p0 + 64, hg, 1, j0:j0 + 128],
                        rhs=qkt[p0:p0 + 64, hg, 0, i0 + off:i0 + 512],
                        start=True, stop=True,
                    )
                nc.scalar.activation(pt[:, 0:2 * w], ps_s[:, 0:2 * w],
                                     AF.Exp)
                if straddle:
                    nc.vector.tensor_tensor(
                        out=pt[:, 0:128], in0=pt[:, 0:128],
                        in1=mask_sb[:, 0:128],
                        op=mybir.AluOpType.mult)
                    nc.vector.tensor_tensor(
                        out=pt[:, w:w + 256],
                        in0=pt[:, w:w + 256],
                        in1=mask_sb[:, 128:384],
                        op=mybir.AluOpType.mult)
                pts[p] = pt

            def y_pair(p):
                pt = pts[p]
                off = max(0, 2 * p * 128 - i0)
                w = 512 - off
                for half in range(2):
                    jj = 2 * p + half
                    nc.tensor.matmul(
                        ps_y[:, off:512],
                        lhsT=vaug[:, :, h * TJ + jj],
                        rhs=pt[:, half * w:half * w + w],
                        start=(jj == 0), stop=(jj == njt - 1),
                    )

            def norm():
                # l -> SBUF f32r (DVE), PE ones-broadcast to 64 lanes,
                # copy out (ACT), reciprocal + multiply (DVE)
                lsb = lpool.tile([D + 1, 512], F32R, tag="l", name="lsb")
                with nc.allow_low_precision(reason="l to f32r for bcast"):
                    nc.vector.tensor_copy(lsb[D:D + 1, :], ps_y[D:D + 1, :])
                ps_b = psY.tile([64, 512], F32, tag="y", name="psb")
                nc.tensor.matmul(ps_b[:, :], lhsT=ones_sb[64:65, :],
                                 rhs=lsb[D:D + 1, :],
                                 start=True, stop=True)
                bcl = rpool.tile([64, 512], F32, tag="r", name="bcl")
                nc.scalar.activation(bcl[:, :], ps_b[:, :], AF.Copy)
                bc = bcpool.tile([64, 512], F32, tag="bc", name="bc")
                nc.vector.reciprocal_approx_fast(out=bc[:, :], in_=bcl[:, :])
                if hh == 0:
                    nc.vector.tensor_mul(yt[0:64, hg, i0:i0 + 512],
                                         ps_y[0:D, :], bc[:, :])
                else:
                    yst = ystpool.tile([64, 512], BF16, tag="yst",
                                       name="yst")
                    nc.vector.tensor_mul(yst[:, :], ps_y[0:D, :], bc[:, :])
                    nc.sync.dma_start(out=yt[64:128, hg, i0:i0 + 512],
                                      in_=yst[:, :])

            return s_pair, y_pair, norm, npair

        # ---------- prologue: batch 0 q/k straight through, v deferred ----
        for u in a_units(0, defer_v=True):
            u()
        for u in v_units(0, 0):
            u()
        for lt in range(1, TI):
            filler.extend(v_units(0, lt))

        # ---------- main loop ----------
        for b in range(BL):
            last = (b + 1 == BL)
            if not last:
                filler.extend(a_units(b + 1))

            for it in range(TI):
                i0 = it * 512
                if not last:
                    # single stream per head; filler hides exp latency
                    for h in range(HPC):
                        s_pair, y_pair, norm, npair = make_stream(b, it, h)
                        s_pair(0)
                        for p in range(1, npair):
                            s_pair(p)
                            pump(4)
                            y_pair(p - 1)
                        pump(3)
                        y_pair(npair - 1)
                        norm()
                else:
                    # last batch: no more QKV filler — interleave two head
                    # streams so the PE rides one while the other waits on exp
                    for hp in range(HPC // 2):
                        sA = make_stream(b, it, 2 * hp)
                        sB = make_stream(b, it, 2 * hp + 1)
                        npair = sA[3]
                        sA[0](0)
                        sB[0](0)
                        for p in range(1, npair):
                            sA[0](p)
                            pump(2)
                            sA[1](p - 1)
                            sB[0](p)
                            pump(2)
                            sB[1](p - 1)
                        sA[1](npair - 1)
                        sB[1](npair - 1)
                        sA[2]()
                        sB[2]()

                # ---- phase C for this i-column: deferred via filler ----
                filler.extend(c_units(b, i0, yts[b]))

        pump(len(filler))

    nc.compile()
    return nc


def make_in_maps(x, Wqkv, bqkv, Wproj, bproj):
    Bx, Tx, Cx = x.shape
    bf = ml_dtypes.bfloat16
    BL = Bx // DPC
    R = BL * Tx
    # per-dp-group transposed activations
    xTh = []
    for dp in range(DPC):
        xg = x[dp * BL:(dp + 1) * BL].reshape(R, Cx)
        xTh.append(np.ascontiguousarray(
            xg.T.astype(np.float32)).astype(bf))
    ident_h = np.eye(128, dtype=np.float32)
    ones64_h = np.ones((65, 64), np.float32)
    S = BL * HPC * (Tx // 128)
    vones_h = np.ones((128, S), np.float32)
    m128 = np.triu(np.ones((128, 128), np.float32))
    m256 = np.zeros((128, 256), np.float32)
    for r in range(128):
        m256[r, 128 + r:] = 1.0
    masks_h = np.concatenate([m128, m256], axis=1).astype(bf)
    assert not np.any(bqkv) and not np.any(bproj), \
        "nonzero biases unsupported in this build"
    in_maps = []
    for i in range(NCORES):
        tp, dp = i % TPC, i // TPC
        cs = slice(tp * FPC, (tp + 1) * FPC)
        wq = Wqkv[:, 0 * C:1 * C][:, cs] * SCALE
        wk = Wqkv[:, 1 * C:2 * C][:, cs]
        wv = Wqkv[:, 2 * C:3 * C][:, cs]
        # ci order: q_hg0, q_hg1, k_hg0, k_hg1, v_hg0, v_hg1
        wqkv_s = np.ascontiguousarray(np.concatenate(
            [wq[:, 0:128], wq[:, 128:256],
             wk[:, 0:128], wk[:, 128:256],
             wv[:, 0:128], wv[:, 128:256]], axis=1)
            .astype(np.float32)).astype(bf)
        wp_s = np.ascontiguousarray(Wproj[cs, :].astype(np.float32)).astype(bf)
        in_maps.append({
            "xT": xTh[dp],
            "wqkv": wqkv_s,
            "wp": wp_s,
            "ident": ident_h.astype(bf),
            "ones64": ones64_h,
            "vones": vones_h.astype(bf),
            "masks": masks_h,
        })
    return in_maps


def kernel(x, Wqkv, bqkv, Wproj, bproj, trace=False):
    global LAST_RESULT
    x = np.asarray(x, dtype=np.float32)
    Wqkv = np.asarray(Wqkv, dtype=np.float32)
    bqkv = np.asarray(bqkv, dtype=np.float32)
    Wproj = np.asarray(Wproj, dtype=np.float32)
    bproj = np.asarray(bproj, dtype=np.float32)
    Bx, Tx, Cx = x.shape
    assert Cx == C

    key = (Bx, Tx)
    if key not in _CACHE:
        _CACHE[key] = build_program(Bx, Tx)
    nc = _CACHE[key]

    in_maps = make_in_maps(x, Wqkv, bqkv, Wproj, bproj)
    res = run_bass_kernel_spmd(nc, in_maps, list(range(NCORES)), trace=trace)
    LAST_RESULT = res
    BL = Bx // DPC
    out = np.empty((Bx, Tx, Cx), dtype=np.float32)
    for dp in range(DPC):
        acc = np.zeros((C, BL * Tx), dtype=np.float32)
        for tp in range(TPC):
            acc += res.results[dp * TPC + tp]["outT"].astype(np.float32)
        out[dp * BL:(dp + 1) * BL] = \
            np.ascontiguousarray(acc.T).reshape(BL, Tx, Cx)
    return out


# revision 21
# speedup vs baseline: 1.3158x; 1.0473x over previous
"""Causal self-attention (B=4, T=2048, C=1024, H=16) on 8 trn2 NeuronCores.

Sharding: 4-way tensor parallel over heads x 2-way data parallel over batch.
Core i handles head group tp = i % 4 (4 heads) for batches [2*dp, 2*dp+1],
dp = i // 4.  Each core:
  - Wqkv columns for its 4 heads ([1024, 768] bf16, q-part pre-scaled 1/8)
  - Wproj rows for its heads ([256, 1024] bf16)
  - x for its 2 batches, transposed to [C, 2T] bf16 on host
Each core computes a partial projection [C, 2T] bf16; the host sums the 4
TP partials per batch pair in fp32 and un-transposes.

All PE operands are bf16 (f32r streams ~1.5-2x slower on HW and fp32
weight loads are 2x slower).  On-core dataflow:
  A) QKV^T per 512-token l-tile: 6 single-bank accumulation passes
     (q/k/v x 2 head-groups); q/k copied to SBUF bf16; v staged bf16 and
     PE-transposed (2 heads per transpose) into V-natural layout vaug
     with a trailing ones row (row sums l for free in PV).
  B) attention per (b, h, 512-wide i-tile): S^T j-tile pairs share a
     2-bank PSUM tile and a common query window (both halves start at the
     even tile's clip offset), ONE exp per pair (bf16 out), causal masking
     via two small bf16 mask multiplies on DVE, Y^T accumulated per j-tile.
     Softmax normalization: ACT copies l to SBUF, DVE fast reciprocal,
     gpsimd partition_broadcast to 64 lanes, one DVE multiply.
  C) output projection per i-tile (2-pass K=256 accumulation, bf16 copy
     out on DVE -> DMA).
  QKV work for batch b+1 and projection work are emitted as fine-grained
  FILLER between attention pairs so the tensor queue never drains.
"""

import numpy as np
from contextlib import ExitStack

import ml_dtypes

import concourse.bacc as bacc
import concourse.bass as bass
import concourse.mybir as mybir
import concourse.tile as tile
from concourse.bass_utils import run_bass_kernel_spmd

NCORES = 8
TPC = 4                # tensor-parallel cores (head groups)
DPC = 2                # data-parallel groups
C = 1024
H = 16
D = 64                 # head dim
HPC = H // TPC         # heads per core = 4
HG = HPC // 2          # head groups of 2 per core = 2
FPC = HPC * D          # features per core = 256
KC = C // 128          # contraction chunks = 8
SCALE = 1.0 / 8.0      # 1/sqrt(D)

F32 = mybir.dt.float32
BF16 = mybir.dt.bfloat16
AF = mybir.ActivationFunctionType

_CACHE = {}
LAST_RESULT = None


def build_program(B, T):
    BL = B // DPC          # local batches = 2
    R = BL * T             # local tokens = 4096
    TJ = T // 128          # 128-wide j (key) tiles per sequence = 16
    TI = T // 512          # 512-wide i (query) tiles per sequence = 4
    SB = HPC * TJ          # vaug stripes per batch = 64
    assert T % 512 == 0

    nc = bacc.Bacc("TRN2", target_bir_lowering=False, debug=False,
                   num_devices=NCORES)
    xT = nc.dram_tensor("xT", [C, R], BF16, kind="ExternalInput").ap()
    wqkv = nc.dram_tensor("wqkv", [C, 3 * FPC], BF16,
                          kind="ExternalInput").ap()
    wp = nc.dram_tensor("wp", [FPC, C], BF16, kind="ExternalInput").ap()
    ident = nc.dram_tensor("ident", [128, 128], BF16,
                           kind="ExternalInput").ap()
    ones64 = nc.dram_tensor("ones64", [65, 64], F32,
                            kind="ExternalInput").ap()
    vones = nc.dram_tensor("vones", [128, BL * SB], BF16,
                           kind="ExternalInput").ap()
    # masks[:, 0:128]  = tril (keep col >= row)
    # masks[:, 128:384] = keep col-128 >= row (odd half of a pair window)
    masks = nc.dram_tensor("masks", [128, 384], BF16,
                           kind="ExternalInput").ap()
    outT = nc.dram_tensor("outT", [C, R], BF16, kind="ExternalOutput").ap()

    with tile.TileContext(nc) as tc, ExitStack() as ctx:
        const = ctx.enter_context(tc.tile_pool(name="const", bufs=1))
        big = ctx.enter_context(tc.tile_pool(name="big", bufs=1))
        xpool = ctx.enter_context(tc.tile_pool(name="xpool", bufs=32))
        vspool = ctx.enter_context(tc.tile_pool(name="vspool", bufs=2))
        ptpool = ctx.enter_context(tc.tile_pool(name="ptpool", bufs=6))
        lpool = ctx.enter_context(tc.tile_pool(name="lpool", bufs=2))
        rpool = ctx.enter_context(tc.tile_pool(name="rpool", bufs=2))
        bcpool = ctx.enter_context(tc.tile_pool(name="bcpool", bufs=2))
        ystpool = ctx.enter_context(tc.tile_pool(name="ystpool", bufs=2))
        opool = ctx.enter_context(tc.tile_pool(name="opool", bufs=3))
        psA = ctx.enter_context(tc.tile_pool(name="psA", bufs=2, space="PSUM"))
        psS = ctx.enter_context(tc.tile_pool(name="psS", bufs=2, space="PSUM"))
        psY = ctx.enter_context(tc.tile_pool(name="psY", bufs=2, space="PSUM"))

        # ---- constants ----
        w_sb = const.tile([128, KC, 3 * FPC], BF16)
        nc.sync.dma_start(out=w_sb,
                          in_=wqkv.rearrange("(kc p) c -> p kc c", p=128))
        wp_sb = const.tile([128, 2, C], BF16)
        nc.sync.dma_start(out=wp_sb,
                          in_=wp.rearrange("(kc p) c -> p kc c", p=128))
        identcol = const.tile([128, 128], BF16)
        nc.sync.dma_start(out=identcol, in_=ident)
        F32R = mybir.dt.float32r
        ones_sb = const.tile([65, 64], F32R)
        nc.sync.dma_start(out=ones_sb, in_=ones64.bitcast(F32R))
        mask_sb = const.tile([128, 384], BF16)
        nc.sync.dma_start(out=mask_sb, in_=masks)

        # per-batch persistent tiles
        qkts, yts, vaugs = [], [], []
        for b in range(BL):
            qkts.append(big.tile([128, HG, 2, T], BF16, name=f"qkt{b}",
                                 tag=f"qkt{b}"))
            yts.append(big.tile([128, 2, T], BF16, name=f"yt{b}",
                                tag=f"yt{b}"))
            v = big.tile([128, D + 1, SB], BF16, name=f"va{b}", tag=f"va{b}")
            nc.sync.dma_start(out=v[:, D, :],
                              in_=vones[:, b * SB:(b + 1) * SB])
            vaugs.append(v)

        xT3 = xT.rearrange("(kc p) r -> p kc r", p=128)
        outTr = outT.rearrange("(cc p) r -> cc p r", p=128)

        # ---------- phase A unit generators (QKV projection) ----------
        xts_store = {}

        def ci_units(b, lt, ci):
            """8 accumulation matmuls + 1 epilogue unit for one 128-col
            weight slice of l-tile (b, lt)."""
            qkt, vaug = qkts[b], vaugs[b]
            l0 = lt * 512
            xts = xts_store[(b, lt)]
            ps_box = []

            def a_mm(k, ci=ci, ps_box=ps_box, xts=xts):
                if not ps_box:
                    ps_box.append(
                        psA.tile([128, 512], F32, tag="a", name="psa"))
                nc.tensor.matmul(
                    ps_box[0][:, :],
                    lhsT=w_sb[:, k, ci * 128:(ci + 1) * 128],
                    rhs=xts[k][:, :],
                    start=(k == 0), stop=(k == KC - 1),
                )

            for k in range(KC):
                yield lambda k=k, f=a_mm: f(k)

            if ci < 4:
                # q/k: ci -> (kind=ci//2, hg=ci%2)
                def a_copy(ci=ci, ps_box=ps_box, l0=l0, qkt=qkt):
                    qk, hg = ci // 2, ci % 2
                    nc.scalar.activation(
                        qkt[:, hg, qk, l0:l0 + 512], ps_box[0][:, :],
                        AF.Copy)

                yield a_copy
            else:
                # v head-group hg = ci-4: stage, transpose, store
                def v_stage(ci=ci, ps_box=ps_box, lt=lt, vaug=vaug):
                    hg = ci - 4
                    vstage = vspool.tile([128, 512], BF16, tag="vs",
                                         name="vstage")
                    nc.scalar.activation(vstage[:, :],
                                         ps_box[0][:, :], AF.Copy)
                    ps_t = psA.tile([128, 512], BF16, tag="a",
                                    name="pst")
                    for jb in range(4):
                        nc.tensor.matmul(
                            ps_t[:, jb * 128:(jb + 1) * 128],
                            lhsT=vstage[:, jb * 128:(jb + 1) * 128],
                            rhs=identcol[:, :],
                            is_transpose=True,
                            start=(jb == 0), stop=(jb == 3),
                            skip_group_check=True,
                        )
                    # stripes: head h=2*hg+hh at (2*hg+hh)*TJ + lt*4
                    va_r = vaug[:, 0:D, :].rearrange(
                        "p d (h t j) -> p d h t j",
                        h=HPC, t=TI, j=4)
                    nc.vector.tensor_copy(
                        va_r[:, :, 2 * hg:2 * hg + 2, lt, :],
                        ps_t.rearrange("p (jb hh d) -> p d hh jb",
                                       jb=4, hh=2))

                yield v_stage

        def v_units(b, lt):
            for ci in (4, 5):
                yield from ci_units(b, lt, ci)

        def a_units(b, defer_v=False):
            """Yield emission closures for batch b's QKV projection."""
            for lt in range(TI):
                r0 = b * T + lt * 512
                xts = xts_store.setdefault((b, lt), [])

                def dma_unit(k, r0=r0, xts=xts):
                    xt = xpool.tile([128, 512], BF16, tag="xt",
                                    name=f"xt{b}_{k}")
                    nc.sync.dma_start(out=xt,
                                      in_=xT3[:, k, r0:r0 + 512])
                    xts.append(xt)

                for k in range(KC):
                    yield lambda k=k, f=dma_unit: f(k)

                for ci in range(4):
                    yield from ci_units(b, lt, ci)
                if not defer_v:
                    yield from v_units(b, lt)

        filler = []

        def pump(n):
            for _ in range(n):
                if not filler:
                    return
                filler.pop(0)()

        # ---------- phase C unit generator (output projection) ----------
        def c_units(b, i0, yt):
            w0 = b * T + i0
            for ct in range(KC):
                box = []

                def c_mm(kc, ct=ct, box=box, yt=yt, i0=i0):
                    if not box:
                        box.append(psA.tile([128, 512], F32, tag="a",
                                            name="pso"))
                    nc.tensor.matmul(
                        box[0][:, :],
                        lhsT=wp_sb[:, kc, ct * 128:(ct + 1) * 128],
                        rhs=yt[:, kc, i0:i0 + 512],
                        start=(kc == 0), stop=(kc == 1),
                    )

                for kc in range(2):
                    yield lambda kc=kc, f=c_mm: f(kc)

                def c_out(ct=ct, box=box, w0=w0):
                    ost = opool.tile([128, 512], BF16, tag="o", name="ost")
                    nc.vector.tensor_copy(ost[:, :], box[0][:, :])
                    nc.sync.dma_start(
                        out=outTr[ct, :, w0:w0 + 512],
                        in_=ost[:, :],
                    )

                yield c_out

        # ---------- phase B stream factory ----------
        def make_stream(b, it, h):
            qkt, yt, vaug = qkts[b], yts[b], vaugs[b]
            i0 = it * 512
            njt = (i0 + 512) // 128
            npair = njt // 2
            hg, hh = h // 2, h % 2
            p0 = 64 * hh
            ps_y = psY.tile([D + 1, 512], F32, tag="y", name="psy")
            pts = [None] * npair

            def s_pair(p):
                # both halves share the even tile's query window; the odd
                # strip is laid contiguously at [w, 2w) so one exp covers
                # exactly the written region
                off = max(0, 2 * p * 128 - i0)
                w = 512 - off
                straddle = (2 * p + 1) * 128 > i0
                ps_s = psS.tile([128, 1024], F32, tag="s", name="pss")
                pt = ptpool.tile([128, 1024], BF16, tag="pt", name="pt")
                for half in range(2):
                    j0 = (2 * p + half) * 128
                    nc.tensor.matmul(
                        ps_s[:, half * w:half * w + w],
                        lhsT=qkt[p0:p0 + 64, hg, 1, j0:j0 + 128],
                        rhs=qkt[p0:p0 + 64, hg, 0, i0 + off:i0 + 512],
                        start=True, stop=True,
                    )
                nc.scalar.activation(pt[:, 0:2 * w], ps_s[:, 0:2 * w],
                                     AF.Exp)
                if straddle:
                    nc.vector.tensor_tensor(
                        out=pt[:, 0:128], in0=pt[:, 0:128],
                        in1=mask_sb[:, 0:128],
                        op=mybir.AluOpType.mult)
                    nc.vector.tensor_tensor(
                        out=pt[:, w:w + 256],
                        in0=pt[:, w:w + 256],
                        in1=mask_sb[:, 128:384],
                        op=mybir.AluOpType.mult)
                pts[p] = pt

            def y_pair(p):
                pt = pts[p]
                off = max(0, 2 * p * 128 - i0)
                w = 512 - off
                for half in range(2):
                    jj = 2 * p + half
                    nc.tensor.matmul(
                        ps_y[:, off:512],
                        lhsT=vaug[:, :, h * TJ + jj],
                        rhs=pt[:, half * w:half * w + w],
                        start=(jj == 0), stop=(jj == njt - 1),
                    )

            def norm():
                # l -> SBUF f32r (DVE), PE ones-broadcast to 64 lanes,
                # copy out (ACT), reciprocal + multiply (DVE)
                lsb = lpool.tile([D + 1, 512], F32R, tag="l", name="lsb")
                with nc.allow_low_precision(reason="l to f32r for bcast"):
                    nc.vector.tensor_copy(lsb[D:D + 1, :], ps_y[D:D + 1, :])
                ps_b = psS.tile([64, 512], F32, tag="s", name="psb")
                nc.tensor.matmul(ps_b[:, :], lhsT=ones_sb[64:65, :],
                                 rhs=lsb[D:D + 1, :],
                                 start=True, stop=True)
                bcl = rpool.tile([64, 512], F32, tag="r", name="bcl")
                nc.scalar.activation(bcl[:, :], ps_b[:, :], AF.Copy)
                bc = bcpool.tile([64, 512], F32, tag="bc", name="bc")
                nc.vector.reciprocal_approx_fast(out=bc[:, :], in_=bcl[:, :])
                if hh == 0:
                    nc.vector.tensor_mul(yt[0:64, hg, i0:i0 + 512],
                                         ps_y[0:D, :], bc[:, :])
                else:
                    yst = ystpool.tile([64, 512], BF16, tag="yst",
                                       name="yst")
                    nc.vector.tensor_mul(yst[:, :], ps_y[0:D, :], bc[:, :])
                    nc.sync.dma_start(out=yt[64:128, hg, i0:i0 + 512],
                                      in_=yst[:, :])

            return s_pair, y_pair, norm, npair

        # ---------- prologue: batch 0 q/k straight through, v deferred ----
        for u in a_units(0, defer_v=True):
            u()
        for u in v_units(0, 0):
            u()
        for lt in range(1, TI):
            filler.extend(v_units(0, lt))

        # ---------- main loop ----------
        for b in range(BL):
            last = (b + 1 == BL)
            if not last:
                filler.extend(a_units(b + 1))

            for it in range(TI):
                i0 = it * 512
                if not last:
                    # single stream per head; filler hides exp latency
                    for h in range(HPC):
                        s_pair, y_pair, norm, npair = make_stream(b, it, h)
                        s_pair(0)
                        for p in range(1, npair):
                            s_pair(p)
                            pump(4)
                            y_pair(p - 1)
                        pump(3)
                        y_pair(npair - 1)
                        norm()
                else:
                    # last batch: no more QKV filler — interleave two head
                    # streams so the PE rides one while the other waits on exp
                    for hp in range(HPC // 2):
                        sA = make_stream(b, it, 2 * hp)
                        sB = make_stream(b, it, 2 * hp + 1)
                        npair = sA[3]
                        sA[0](0)
                        sB[0](0)
                        for p in range(1, npair):
                            sA[0](p)
                            pump(2)
                            sA[1](p - 1)
                            sB[0](p)
                            pump(2)
                            sB[1](p - 1)
                        sA[1](npair - 1)
                        sB[1](npair - 1)
                        sA[2]()
                        sB[2]()

                # ---- phase C for this i-column: deferred via filler ----
                filler.extend(c_units(b, i0, yts[b]))

        pump(len(filler))

    nc.compile()
    return nc


def make_in_maps(x, Wqkv, bqkv, Wproj, bproj):
    Bx, Tx, Cx = x.shape
    bf = ml_dtypes.bfloat16
    BL = Bx // DPC
    R = BL * Tx
    # per-dp-group transposed activations
    xTh = []
    for dp in range(DPC):
        xg = x[dp * BL:(dp + 1) * BL].reshape(R, Cx)
        xTh.append(np.ascontiguousarray(
            xg.T.astype(np.float32)).astype(bf))
    ident_h = np.eye(128, dtype=np.float32)
    ones64_h = np.ones((65, 64), np.float32)
    S = BL * HPC * (Tx // 128)
    vones_h = np.ones((128, S), np.float32)
    m128 = np.triu(np.ones((128, 128), np.float32))
    m256 = np.zeros((128, 256), np.float32)
    for r in range(128):
        m256[r, 128 + r:] = 1.0
    masks_h = np.concatenate([m128, m256], axis=1).astype(bf)
    assert not np.any(bqkv) and not np.any(bproj), \
        "nonzero biases unsupported in this build"
    in_maps = []
    for i in range(NCORES):
        tp, dp = i % TPC, i // TPC
        cs = slice(tp * FPC, (tp + 1) * FPC)
        wq = Wqkv[:, 0 * C:1 * C][:, cs] * SCALE
        wk = Wqkv[:, 1 * C:2 * C][:, cs]
        wv = Wqkv[:, 2 * C:3 * C][:, cs]
        # ci order: q_hg0, q_hg1, k_hg0, k_hg1, v_hg0, v_hg1
        wqkv_s = np.ascontiguousarray(np.concatenate(
            [wq[:, 0:128], wq[:, 128:256],
             wk[:, 0:128], wk[:, 128:256],
             wv[:, 0:128], wv[:, 128:256]], axis=1)
            .astype(np.float32)).astype(bf)
        wp_s = np.ascontiguousarray(Wproj[cs, :].astype(np.float32)).astype(bf)
        in_maps.append({
            "xT": xTh[dp],
            "wqkv": wqkv_s,
            "wp": wp_s,
            "ident": ident_h.astype(bf),
            "ones64": ones64_h,
            "vones": vones_h.astype(bf),
            "masks": masks_h,
        })
    return in_maps


def kernel(x, Wqkv, bqkv, Wproj, bproj, trace=False):
    global LAST_RESULT
    x = np.asarray(x, dtype=np.float32)
    Wqkv = np.asarray(Wqkv, dtype=np.float32)
    bqkv = np.asarray(bqkv, dtype=np.float32)
    Wproj = np.asarray(Wproj, dtype=np.float32)
    bproj = np.asarray(bproj, dtype=np.float32)
    Bx, Tx, Cx = x.shape
    assert Cx == C

    key = (Bx, Tx)
    if key not in _CACHE:
        _CACHE[key] = build_program(Bx, Tx)
    nc = _CACHE[key]

    in_maps = make_in_maps(x, Wqkv, bqkv, Wproj, bproj)
    res = run_bass_kernel_spmd(nc, in_maps, list(range(NCORES)), trace=trace)
    LAST_RESULT = res
    BL = Bx // DPC
    out = np.empty((Bx, Tx, Cx), dtype=np.float32)
    for dp in range(DPC):
        acc = np.zeros((C, BL * Tx), dtype=np.float32)
        for tp in range(TPC):
            acc += res.results[dp * TPC + tp]["outT"].astype(np.float32)
        out[dp * BL:(dp + 1) * BL] = \
            np.ascontiguousarray(acc.T).reshape(BL, Tx, Cx)
    return out
